# revision 2
# baseline (speedup 1.0000x reference)
"""Trainium2 Bass kernel for causal self-attention with RoPE (nn_CausalSelfAttention).

Problem (hardcoded): B=2, S=2048, D=1024, H=16 heads, head_dim=64, fp32,
causal mask, RoPE (rotate-half, base 10000), torch-Linear projections
q = x @ Wq.T, kv = x @ Wkv.T interleaved (k even, v odd output channels).

Sharding: 8 cores = 2 batches x 4 head-groups (4 heads each, as 2 row-packed
pairs). Everything per-core is local; no collectives.

Device-side layout choices:
  - All projection activations x are fed transposed (d_in on partitions).
  - q,k are produced TRANSPOSED per head-pair: (128 partitions = 2 heads x 64
    dims, seq free) -- this is directly the scores lhsT/rhs layout.
  - Head dims are permuted on partitions ("paired d-order") so the RoPE
    rotate-half partner is always +16 mod 32 within a 32-partition quadrant,
    implementable with a single DVE stream_shuffle.
  - Scores are computed transposed S^T[k, q] per 128-k-chunk with 2 heads
    row-packed in the 128x128 PE array (contraction=64 each).
  - softmax without max-subtraction (scores ~ N(0,1), |s|<~7 -- safe in fp32);
    exp on ScalarE reads PSUM and writes f32r P^T to SBUF.
  - AV: out^T[d, q] accumulated over k-chunks in PSUM; v carries an extra
    ones-column so row 64 accumulates sum(exp) for free.
  - Normalization + final transpose on host (cheap numpy) from the returned
    (heads, 65, S) tensor.
"""

import numpy as np

B, S, D = 2, 2048, 1024
H, HD = 16, 64
NCORES = 8
ROPE_BASE = 10000.0
NKC = D // 128          # contraction chunks for projections (8)
NSC = S // 128          # seq chunks of 128 (16)
NQB = S // 512          # q blocks of 512 (4)

_CACHE = {}


# --------------------------------------------------------------------------
# host-side index maps
# --------------------------------------------------------------------------
def _dperm():
    """Row r (0..63) -> head-dim d, arranged so the rotate-half partner of the
    dim at row r sits at row (r//32)*32 + (r%32+16)%32 (same quadrant)."""
    p = np.empty(64, np.int64)
    for r in range(64):
        quad, i = divmod(r, 32)
        p[r] = 16 * quad + i if i < 16 else 32 + 16 * quad + (i - 16)
    return p


def _rope_tables():
    inv = 1.0 / (ROPE_BASE ** (np.arange(0, HD, 2, dtype=np.float64) / HD))  # (32,)
    t = np.arange(S, dtype=np.float64)
    fr = t[:, None] * inv[None, :]                    # (S, 32)
    return np.cos(fr), np.sin(fr)                     # float64 (S, 32)


# --------------------------------------------------------------------------
# device kernel builder (same NEFF for all 8 cores)
# --------------------------------------------------------------------------
def _build():
    if "nc" in _CACHE:
        return _CACHE["nc"]
    import concourse.tile as tile
    from concourse import bacc, mybir

    f32 = mybir.dt.float32
    f32r = mybir.dt.float32r
    EXP = mybir.ActivationFunctionType.Exp
    MUL = mybir.AluOpType.mult

    nc = bacc.Bacc("TRN2", target_bir_lowering=False, debug=False)
    xT = nc.dram_tensor("xT", [NKC, 128, S], f32r, kind="ExternalInput").ap()
    wq = nc.dram_tensor("wq", [NKC, 128, 256], f32r, kind="ExternalInput").ap()
    wk = nc.dram_tensor("wk", [NKC, 128, 256], f32r, kind="ExternalInput").ap()
    wv = nc.dram_tensor("wv", [NKC, 128, 256], f32r, kind="ExternalInput").ap()
    cosT = nc.dram_tensor("cosT", [128, S], f32, kind="ExternalInput").ap()
    sinT = nc.dram_tensor("sinT", [128, S], f32, kind="ExternalInput").ap()
    tri = nc.dram_tensor("tri", [128, 128], f32r, kind="ExternalInput").ap()
    vones = nc.dram_tensor("vones", [128, NSC, 4], f32r, kind="ExternalInput").ap()
    o = nc.dram_tensor("o", [4, 65, S], f32, kind="ExternalOutput").ap()

    shuf_mask = [(i + 16) % 32 for i in range(32)]

    with tile.TileContext(nc) as tc:
        with (
            tc.tile_pool(name="cst", bufs=1) as cst,
            tc.tile_pool(name="rope", bufs=3) as rope,
            tc.tile_pool(name="ptp", bufs=4) as ptp,
            tc.tile_pool(name="ost", bufs=3) as ost,
        ):
            xT_sb = cst.tile([128, NKC, S], f32r, tag="xT")
            wq_sb = cst.tile([128, NKC, 256], f32r, tag="wq")
            wk_sb = cst.tile([128, NKC, 256], f32r, tag="wk")
            wv_sb = cst.tile([128, NKC, 256], f32r, tag="wv")
            cos_sb = cst.tile([128, S], f32, tag="cos")
            sin_sb = cst.tile([128, S], f32, tag="sin")
            tri_sb = cst.tile([128, 128], f32r, tag="tri")
            qT_sb = cst.tile([128, 2, S], f32r, tag="qT")
            kT_sb = cst.tile([128, 2, S], f32r, tag="kT")
            vx_sb = cst.tile([128, NSC, 4, 65], f32r, tag="vx")

            for kc in range(NKC):
                nc.sync.dma_start(xT_sb[:, kc, :], xT[kc])
                nc.sync.dma_start(wq_sb[:, kc, :], wq[kc])
                nc.sync.dma_start(wk_sb[:, kc, :], wk[kc])
                nc.sync.dma_start(wv_sb[:, kc, :], wv[kc])
            nc.sync.dma_start(cos_sb[:], cosT)
            nc.sync.dma_start(sin_sb[:], sinT)
            nc.sync.dma_start(tri_sb[:], tri)
            # ones-columns (index 64 of each head slot); v copies leave them.
            nc.sync.dma_start(vx_sb[:, :, :, 64], vones)

            # ---------------- projections: q, k (+RoPE fused) ----------------
            with tc.tile_pool(name="pps", bufs=3, space="PSUM") as pps:
                for dst, w_sb in ((qT_sb, wq_sb), (kT_sb, wk_sb)):
                    for t in range(2):          # head pair tile
                        for sb in range(4):     # 512-wide seq block
                            ps = pps.tile([128, 512], f32, tag="proj", name=f"ps_{t}_{sb}")
                            for kc in range(NKC):
                                nc.tensor.matmul(
                                    ps[:],
                                    w_sb[:, kc, t * 128:(t + 1) * 128],
                                    xT_sb[:, kc, sb * 512:(sb + 1) * 512],
                                    start=(kc == 0), stop=(kc == NKC - 1))
                            sl = slice(sb * 512, (sb + 1) * 512)
                            shf = rope.tile([128, 512], f32, tag="shf")
                            nc.vector.stream_shuffle(shf[:], ps[:], shuf_mask)
                            m2 = rope.tile([128, 512], f32, tag="m2")
                            nc.vector.tensor_tensor(m2[:], shf[:], sin_sb[:, sl], MUL)
                            m1 = rope.tile([128, 512], f32, tag="m1")
                            nc.vector.tensor_tensor(m1[:], ps[:], cos_sb[:, sl], MUL)
                            nc.vector.tensor_add(dst[:, t, sl], m1[:], m2[:])

                # ---------------- projection: v (natural layout) -------------
                for sc in range(NSC):
                    psv = pps.tile([128, 256], f32, tag="projv", name=f"psv_{sc}")
                    for kc in range(NKC):
                        nc.tensor.matmul(
                            psv[:],
                            xT_sb[:, kc, sc * 128:(sc + 1) * 128],
                            wv_sb[:, kc, :],
                            start=(kc == 0), stop=(kc == NKC - 1))
                    nc.vector.tensor_copy(
                        vx_sb[:, sc, :, 0:64],
                        psv[:].rearrange("p (h d) -> p h d", h=4))

            # ---------------- attention ----------------
            with (
                tc.tile_pool(name="scp", bufs=2, space="PSUM") as scp,
                tc.tile_pool(name="ops", bufs=2, space="PSUM") as ops,
            ):
                for pair in range(2):
                    for qb in range(NQB):
                        qlo = qb * 512
                        o_ps = [ops.tile([65, 512], f32, tag=f"o{h}",
                                         name=f"o_ps{pair}_{qb}_{h}")
                                for h in range(2)]
                        nchunks = 4 * qb + 4
                        for c in range(nchunks):
                            s = c - 4 * qb        # >=0 on diagonal chunks
                            sc_t = scp.tile([128, 2, 512], f32, tag="sc",
                                            name=f"sc_{pair}_{qb}_{c}")
                            for h in range(2):
                                nc.tensor.matmul(
                                    sc_t[:, h, :],
                                    kT_sb[h * 64:(h + 1) * 64, pair,
                                          c * 128:(c + 1) * 128],
                                    qT_sb[h * 64:(h + 1) * 64, pair,
                                          qlo:qlo + 512],
                                    start=True, stop=True)
                            lo = 0 if s < 0 else 128 * s
                            pt = ptp.tile([128, 2, 512], f32r, tag="pt")
                            nc.scalar.activation(
                                pt[:, :, lo:], sc_t[:, :, lo:], EXP, scale=0.125)
                            if s >= 0:
                                nc.vector.tensor_tensor(
                                    pt[:, :, lo:lo + 128],
                                    pt[:, :, lo:lo + 128],
                                    tri_sb[:].unsqueeze(1).broadcast_to(
                                        [128, 2, 128]),
                                    MUL)
                            for h in range(2):
                                nc.tensor.matmul(
                                    o_ps[h][:, lo:512],
                                    vx_sb[:, c, 2 * pair + h, :],
                                    pt[:, h, lo:512],
                                    start=(c == 0), stop=(c == nchunks - 1))
                        for h in range(2):
                            o_sb = ost.tile([65, 512], f32, tag="ost")
                            nc.vector.tensor_copy(o_sb[:], o_ps[h][:])
                            nc.sync.dma_start(
                                o[2 * pair + h, :, qlo:qlo + 512], o_sb[:])

    nc.compile()
    _CACHE["nc"] = nc
    return nc


# --------------------------------------------------------------------------
# host-side sharding / unsharding
# --------------------------------------------------------------------------
def _make_in_maps(x, Wq, Wkv):
    x = np.asarray(x, np.float32)
    Wq = np.asarray(Wq, np.float32)
    Wkv = np.asarray(Wkv, np.float32)

    dp = _dperm()
    cos32, sin32 = _rope_tables()
    sign = np.where((np.arange(128) % 32) < 16, -1.0, 1.0)
    rows64 = np.concatenate([dp, dp])                       # 128 rows, 2 heads
    cosT = cos32[:, rows64 % 32].T.astype(np.float32)       # (128, S)
    sinT = (sin32[:, rows64 % 32].T * sign[:, None]).astype(np.float32)
    tri = (np.arange(128)[:, None] <= np.arange(128)[None, :]).astype(np.float32)

    xT_b = [np.ascontiguousarray(x[b].T).reshape(NKC, 128, S) for b in range(B)]

    in_maps = []
    for c in range(NCORES):
        b, g = divmod(c, 4)
        heads = [4 * g + hh for hh in range(4)]
        qrows = np.concatenate([h * 64 + dp for h in heads])
        krows = np.concatenate([h * 128 + 2 * dp for h in heads])
        vrows = np.concatenate([h * 128 + 2 * np.arange(64) + 1 for h in heads])
        wq_c = np.ascontiguousarray(Wq[qrows, :].T).reshape(NKC, 128, 256)
        wk_c = np.ascontiguousarray(Wkv[krows, :].T).reshape(NKC, 128, 256)
        wv_c = np.ascontiguousarray(Wkv[vrows, :].T).reshape(NKC, 128, 256)
        in_maps.append({
            "xT": xT_b[b], "wq": wq_c, "wk": wk_c, "wv": wv_c,
            "cosT": cosT, "sinT": sinT, "tri": tri,
            "vones": np.ones((128, NSC, 4), np.float32),
        })
    return in_maps


def _assemble(results):
    out = np.empty((B, S, D), np.float32)
    for c in range(NCORES):
        b, g = divmod(c, 4)
        oc = results[c]["o"]                        # (4, 65, S)
        att = oc[:, :64, :] / oc[:, 64:65, :]       # (4, 64, S)
        for hh in range(4):
            head = 4 * g + hh
            out[b, :, head * 64:(head + 1) * 64] = att[hh].T
    return out


def kernel(x, Wq, Wkv, mask=None):
    from concourse.bass_utils import run_bass_kernel_spmd

    nc = _build()
    in_maps = _make_in_maps(x, Wq, Wkv)
    res = run_bass_kernel_spmd(nc, in_maps, core_ids=list(range(NCORES)))
    return _assemble(res.results)


# revision 6
# speedup vs baseline: 556.1299x; 556.1299x over previous
"""Trainium2 Bass kernel for causal self-attention with RoPE (nn_CausalSelfAttention).

Problem (hardcoded): B=2, S=2048, D=1024, H=16 heads, head_dim=64, fp32,
causal mask, RoPE (rotate-half, base 10000), torch-Linear projections
q = x @ Wq.T, kv = x @ Wkv.T interleaved (k even, v odd output channels).

Sharding: 8 cores = 2 batches x 4 head-groups (4 heads each, as 2 row-packed
pairs). Everything per-core is local; no collectives.

Device-side layout choices:
  - All projection activations x are fed transposed (d_in on partitions).
  - q,k are produced TRANSPOSED per head-pair: (128 partitions = 2 heads x 64
    dims, seq free) -- this is directly the scores lhsT/rhs layout.
  - Head dims are permuted on partitions ("paired d-order") so the RoPE
    rotate-half partner is always +16 mod 32 within a 32-partition quadrant,
    implementable with a single DVE stream_shuffle.
  - Scores are computed transposed S^T[k, q] per 128-k-chunk with 2 heads
    row-packed in the 128x128 PE array (contraction=64 each).
  - softmax without max-subtraction (scores ~ N(0,1), |s|<~7 -- safe in fp32);
    exp on ScalarE reads PSUM and writes f32r P^T to SBUF.
  - AV: out^T[d, q] accumulated over k-chunks in PSUM; v carries an extra
    ones-column so row 64 accumulates sum(exp) for free.
  - Normalization + final transpose on host (cheap numpy) from the returned
    (heads, 65, S) tensor.
"""

import numpy as np

B, S, D = 2, 2048, 1024
H, HD = 16, 64
NCORES = 8
ROPE_BASE = 10000.0
NKC = D // 128          # contraction chunks for projections (8)
NSC = S // 128          # seq chunks of 128 (16)
NQB = S // 512          # q blocks of 512 (4)

_CACHE = {}


# --------------------------------------------------------------------------
# host-side index maps
# --------------------------------------------------------------------------
def _dperm():
    """Row r (0..63) -> head-dim d, arranged so the rotate-half partner of the
    dim at row r sits at row (r//32)*32 + (r%32+16)%32 (same quadrant)."""
    p = np.empty(64, np.int64)
    for r in range(64):
        quad, i = divmod(r, 32)
        p[r] = 16 * quad + i if i < 16 else 32 + 16 * quad + (i - 16)
    return p


def _rope_tables():
    inv = 1.0 / (ROPE_BASE ** (np.arange(0, HD, 2, dtype=np.float64) / HD))  # (32,)
    t = np.arange(S, dtype=np.float64)
    fr = t[:, None] * inv[None, :]                    # (S, 32)
    return np.cos(fr), np.sin(fr)                     # float64 (S, 32)


# --------------------------------------------------------------------------
# device kernel builder (same NEFF for all 8 cores)
# --------------------------------------------------------------------------
def _build(reps=1):
    key = ("nc", reps)
    if key in _CACHE:
        return _CACHE[key]
    import concourse.tile as tile
    from concourse import bacc, mybir

    f32 = mybir.dt.float32
    f32r = mybir.dt.float32r
    EXP = mybir.ActivationFunctionType.Exp
    MUL = mybir.AluOpType.mult

    nc = bacc.Bacc("TRN2", target_bir_lowering=False, debug=False)
    xT = nc.dram_tensor("xT", [NKC, 128, S], f32r, kind="ExternalInput").ap()
    wq = nc.dram_tensor("wq", [NKC, 128, 256], f32r, kind="ExternalInput").ap()
    wk = nc.dram_tensor("wk", [NKC, 128, 256], f32r, kind="ExternalInput").ap()
    wv = nc.dram_tensor("wv", [NKC, 128, 256], f32r, kind="ExternalInput").ap()
    cosT = nc.dram_tensor("cosT", [128, S], f32, kind="ExternalInput").ap()
    sinT = nc.dram_tensor("sinT", [128, S], f32, kind="ExternalInput").ap()
    tri = nc.dram_tensor("tri", [128, 128], f32r, kind="ExternalInput").ap()
    vones = nc.dram_tensor("vones", [128, NSC, 4], f32r, kind="ExternalInput").ap()
    o = nc.dram_tensor("o", [4, 65, S], f32, kind="ExternalOutput").ap()

    shuf_mask = [(i + 16) % 32 for i in range(32)]

    with tile.TileContext(nc) as tc:
        with (
            tc.tile_pool(name="cst", bufs=1) as cst,
            tc.tile_pool(name="rope", bufs=3) as rope,
            tc.tile_pool(name="ptp", bufs=4) as ptp,
            tc.tile_pool(name="ost", bufs=3) as ost,
            tc.tile_pool(name="pps", bufs=2, space="PSUM") as pps,
            tc.tile_pool(name="scp", bufs=2, space="PSUM") as scp,
            tc.tile_pool(name="ops", bufs=1, space="PSUM") as ops,
        ):
            xT_sb = cst.tile([128, NKC, S], f32r, tag="xT")
            wq_sb = cst.tile([128, NKC, 256], f32r, tag="wq")
            wk_sb = cst.tile([128, NKC, 256], f32r, tag="wk")
            wv_sb = cst.tile([128, NKC, 256], f32r, tag="wv")
            cos_sb = cst.tile([128, S], f32, tag="cos")
            sin_sb = cst.tile([128, S], f32, tag="sin")
            tri_sb = cst.tile([128, 128], f32r, tag="tri")
            qT_sb = cst.tile([128, 2, S], f32r, tag="qT")
            kT_sb = cst.tile([128, 2, S], f32r, tag="kT")
            vx_sb = cst.tile([128, NSC, 4, 65], f32r, tag="vx")

            def proj_qk_sb(dst, w_sb, t, sb, rp):
                """Project one 512-seq block of one head-pair (q or k) + RoPE."""
                ps = pps.tile([128, 512], f32, tag="proj",
                              name=f"ps_{rp}_{id(dst)}_{t}_{sb}")
                for kc in range(NKC):
                    nc.tensor.matmul(
                        ps[:],
                        w_sb[:, kc, t * 128:(t + 1) * 128],
                        xT_sb[:, kc, sb * 512:(sb + 1) * 512],
                        start=(kc == 0), stop=(kc == NKC - 1))
                sl = slice(sb * 512, (sb + 1) * 512)
                shf = rope.tile([128, 512], f32, tag="shf")
                nc.vector.stream_shuffle(shf[:], ps[:], shuf_mask)
                m2 = rope.tile([128, 512], f32, tag="m2")
                nc.gpsimd.tensor_tensor(m2[:], shf[:], sin_sb[:, sl], MUL)
                m1 = rope.tile([128, 512], f32, tag="m1")
                nc.vector.tensor_tensor(m1[:], ps[:], cos_sb[:, sl], MUL)
                nc.vector.tensor_add(dst[:, t, sl], m1[:], m2[:])

            def proj_v_sc(sc, rp):
                psv = pps.tile([128, 256], f32, tag="proj",
                               name=f"psv_{rp}_{sc}")
                for kc in range(NKC):
                    nc.tensor.matmul(
                        psv[:],
                        xT_sb[:, kc, sc * 128:(sc + 1) * 128],
                        wv_sb[:, kc, :],
                        start=(kc == 0), stop=(kc == NKC - 1))
                nc.vector.tensor_copy(
                    vx_sb[:, sc, :, 0:64],
                    psv[:].rearrange("p (h d) -> p h d", h=4))

            def attn_qb(pair, qb, rp):
                qlo = qb * 512
                o_ps = [ops.tile([65, 512], f32, tag=f"o{h}",
                                 name=f"o_ps{rp}_{pair}_{qb}_{h}")
                        for h in range(2)]
                nchunks = 4 * qb + 4
                for c in range(nchunks):
                    s = c - 4 * qb        # >=0 on diagonal chunks
                    sc_t = scp.tile([128, 2, 512], f32, tag="sc",
                                    name=f"sc_{rp}_{pair}_{qb}_{c}")
                    for h in range(2):
                        nc.tensor.matmul(
                            sc_t[:, h, :],
                            kT_sb[h * 64:(h + 1) * 64, pair,
                                  c * 128:(c + 1) * 128],
                            qT_sb[h * 64:(h + 1) * 64, pair,
                                  qlo:qlo + 512],
                            start=True, stop=True)
                    lo = 0 if s < 0 else 128 * s
                    pt = ptp.tile([128, 2, 512], f32r, tag="pt")
                    nc.scalar.activation(
                        pt[:, :, lo:], sc_t[:, :, lo:], EXP, scale=0.125)
                    if s >= 0:
                        nc.vector.tensor_tensor(
                            pt[:, :, lo:lo + 128],
                            pt[:, :, lo:lo + 128],
                            tri_sb[:].unsqueeze(1).broadcast_to(
                                [128, 2, 128]),
                            MUL)
                    for h in range(2):
                        nc.tensor.matmul(
                            o_ps[h][:, lo:512],
                            vx_sb[:, c, 2 * pair + h, :],
                            pt[:, h, lo:512],
                            start=(c == 0), stop=(c == nchunks - 1))
                for h in range(2):
                    o_sb = ost.tile([65, 512], f32, tag="ost")
                    nc.vector.tensor_copy(o_sb[:], o_ps[h][:])
                    nc.sync.dma_start(
                        o[2 * pair + h, :, qlo:qlo + 512], o_sb[:])

            for rp in range(reps):
                for kc in range(NKC):
                    nc.sync.dma_start(xT_sb[:, kc, :], xT[kc])
                    nc.sync.dma_start(wq_sb[:, kc, :], wq[kc])
                    nc.sync.dma_start(wk_sb[:, kc, :], wk[kc])
                    nc.sync.dma_start(wv_sb[:, kc, :], wv[kc])
                nc.sync.dma_start(cos_sb[:], cosT)
                nc.sync.dma_start(sin_sb[:], sinT)
                nc.sync.dma_start(tri_sb[:], tri)
                # ones-columns (index 64 of each head slot); v copies leave them
                nc.sync.dma_start(vx_sb[:, :, :, 64], vones)

                # Interleaved schedule: attention q-blocks start as soon as
                # their q/k seq-block and v chunks are projected, keeping
                # ScalarE (the long pole) busy from ~10us onward and filling
                # PE stalls with later projections.
                for sb in range(4):
                    proj_qk_sb(qT_sb, wq_sb, 0, sb, rp)
                    proj_qk_sb(kT_sb, wk_sb, 0, sb, rp)
                    for sc in range(4 * sb, 4 * sb + 4):
                        proj_v_sc(sc, rp)
                    attn_qb(0, sb, rp)
                for sb in range(4):
                    proj_qk_sb(qT_sb, wq_sb, 1, sb, rp)
                    proj_qk_sb(kT_sb, wk_sb, 1, sb, rp)
                    attn_qb(1, sb, rp)

    nc.compile()
    _CACHE[key] = nc
    return nc


# --------------------------------------------------------------------------
# host-side sharding / unsharding
# --------------------------------------------------------------------------
def _make_in_maps(x, Wq, Wkv):
    x = np.asarray(x, np.float32)
    Wq = np.asarray(Wq, np.float32)
    Wkv = np.asarray(Wkv, np.float32)

    dp = _dperm()
    cos32, sin32 = _rope_tables()
    sign = np.where((np.arange(128) % 32) < 16, -1.0, 1.0)
    rows64 = np.concatenate([dp, dp])                       # 128 rows, 2 heads
    cosT = cos32[:, rows64 % 32].T.astype(np.float32)       # (128, S)
    sinT = (sin32[:, rows64 % 32].T * sign[:, None]).astype(np.float32)
    tri = (np.arange(128)[:, None] <= np.arange(128)[None, :]).astype(np.float32)

    xT_b = [np.ascontiguousarray(x[b].T).reshape(NKC, 128, S) for b in range(B)]

    in_maps = []
    for c in range(NCORES):
        b, g = divmod(c, 4)
        heads = [4 * g + hh for hh in range(4)]
        qrows = np.concatenate([h * 64 + dp for h in heads])
        krows = np.concatenate([h * 128 + 2 * dp for h in heads])
        vrows = np.concatenate([h * 128 + 2 * np.arange(64) + 1 for h in heads])
        wq_c = np.ascontiguousarray(Wq[qrows, :].T).reshape(NKC, 128, 256)
        wk_c = np.ascontiguousarray(Wkv[krows, :].T).reshape(NKC, 128, 256)
        wv_c = np.ascontiguousarray(Wkv[vrows, :].T).reshape(NKC, 128, 256)
        in_maps.append({
            "xT": xT_b[b], "wq": wq_c, "wk": wk_c, "wv": wv_c,
            "cosT": cosT, "sinT": sinT, "tri": tri,
            "vones": np.ones((128, NSC, 4), np.float32),
        })
    return in_maps


def _assemble(results):
    out = np.empty((B, S, D), np.float32)
    for c in range(NCORES):
        b, g = divmod(c, 4)
        oc = results[c]["o"]                        # (4, 65, S)
        att = oc[:, :64, :] / oc[:, 64:65, :]       # (4, 64, S)
        for hh in range(4):
            head = 4 * g + hh
            out[b, :, head * 64:(head + 1) * 64] = att[hh].T
    return out


def kernel(x, Wq, Wkv, mask=None):
    from concourse.bass_utils import run_bass_kernel_spmd

    nc = _build()
    in_maps = _make_in_maps(x, Wq, Wkv)
    res = run_bass_kernel_spmd(nc, in_maps, core_ids=list(range(NCORES)))
    return _assemble(res.results)
